# revision 1
# baseline (speedup 1.0000x reference)
"""DeepEmbedAttention TRN2 kernel — 8-core SPMD.

Sharding: 2 cores per batch (B=4). Each core computes the full k/v chain for
its batch (T=2048) and attention outputs for 4 query chunks of 256 tokens.
Chunk assignment is causally load-balanced: even cores take chunks {0,3,4,7},
odd cores {1,2,5,6}. The single SPMD program processes chunks at canonical
slot positions; everything position-dependent (q columns, causal masks,
chunk-boundary tokens) arrives as per-core input data, so one program serves
all 8 cores. Softmax needs no max-subtraction: scores are tanh-capped to
[-64, 64], so exp() cannot overflow fp32.

Engine plan: token-shift is done with PE matmuls against constant
superdiagonal/boundary selector matrices (DMA-free). DMAs are batched large
and spread over the three issuing queues (sync = x/vemb1 input streams,
scalar = kemb/vemb2, gpsimd = constants + output stores); SBUF-only
elementwise work is offloaded to the otherwise-idle GpSimd engine. The
output is stored bf16 and widened on host (halves output HBM traffic);
layernorm rsqrt is a magic-constant seed + one Newton step on DVE (keeps
Sqrt off ACT's tanh/exp table; measured rel err 6.7e-3 vs the 2e-2 gate).

Notes from optimization attempts kept for posterity: GPSIMD cannot touch
PSUM on real hardware; tensor_tensor_reduce hangs the DVE; PSUM allows one
pending accumulation group per tile; batching per-tile rsqrt/LN work into
epochs serializes worse on hardware than this per-tile pipeline despite
fewer instructions.
"""

import sys

if "/opt/trn_rl_repo" not in sys.path:
    sys.path.insert(0, "/opt/trn_rl_repo")

import numpy as np

B, T, C = 4, 2048, 1024
QD, KV = 256, 32
SCORE_SCALE, CAP_SCALE = 1024.0, 64.0
EPS = 1e-5
N_CORES = 8
P = 128
CHUNK = 256
NSLOT = 4                       # q-chunks per core
TQ = NSLOT * CHUNK              # 1024 canonical query tokens per core
NT = T // P                     # 16 token tiles (full sequence)
NQT = TQ // P                   # 8 canonical query token tiles
CHUNKS = [[0, 3, 4, 7], [1, 2, 5, 6]]   # parity -> global chunk ids
R = [4, 8, 12, 16]              # k-tiles per slot (max over parities)
MINQS = [0, 512, 1024, 1536]    # min chunk start over parities, per slot
NEED_MASK = [(s, kt) for s in range(NSLOT) for kt in range(R[s])
             if P * (kt + 1) > MINQS[s]]
MASK_IDX = {sk: i for i, sk in enumerate(NEED_MASK)}
NMASK = len(NEED_MASK)          # 16
NEG = -1.0e30


def _build_program(nc, tc, a, apply_gb, bf16, nrep=1, phases=4):
    from contextlib import ExitStack

    import concourse.mybir as mybir
    from concourse.masks import make_identity

    f32 = mybir.dt.float32
    DT = mybir.dt.bfloat16 if bf16 else f32
    NMAX = 512                      # psum-bank limit caps matmul free size
    Alu = mybir.AluOpType
    Act = mybir.ActivationFunctionType

    xTr = a["xT"].rearrange("(a p) t -> p a t", p=P)        # [128, 8, 2048]
    xqTr = a["xqT"].rearrange("(a p) t -> p a t", p=P)      # [128, 8, 1024]
    xqpr = a["xqprevT"].rearrange("(a p) t -> p a t", p=P)  # [128, 8, 4]
    wqqr = a["wqq"].rearrange("(a p) d -> p a d", p=P)      # [128, 8, 256]
    wkvr = a["wkv"].rearrange("(a p) d -> p a d", p=P)      # [128, 8, 64]
    kembr = a["kemb"].rearrange("(g p) d -> p g d", p=P)    # [128, 16, 256]
    vembr1 = a["vemb1"].rearrange("(g p) d -> p g d", p=P)  # [128, 16, 1024]
    vembr2 = a["vemb2"].rearrange("(g p) d -> p g d", p=P)
    maskr = a["mask"].rearrange("m p q -> p m q")           # [128, 16, 256]
    out_d = a["out"]                                        # [1024, 1024]

    ctx = ExitStack()
    const = ctx.enter_context(tc.tile_pool(name="const", bufs=1))
    pers = ctx.enter_context(tc.tile_pool(name="pers", bufs=1))

    # --- constants (gpsimd queue for the DMAs) ---
    ident = const.tile([P, P], DT, tag="ident")
    make_identity(nc, ident[:])
    # ssup[p, m] = 1 iff m == p+1 : shift-down-one (sh[m] = v[m-1])
    ssup = const.tile([P, P], DT, tag="ssup")
    nc.gpsimd.memset(ssup[:], 0.0)
    nc.gpsimd.affine_select(out=ssup[:], in_=ssup[:],
                            compare_op=Alu.not_equal, fill=1.0,
                            base=1, pattern=[[-1, P]], channel_multiplier=1)
    # bnd[p, m] = 1 iff (p==127, m==0) : carry prev tile's last row into row 0
    bnd = const.tile([P, P], DT, tag="bnd")
    nc.gpsimd.memset(bnd[:], 0.0)
    nc.gpsimd.affine_select(out=bnd[:], in_=bnd[:],
                            compare_op=Alu.not_equal, fill=1.0,
                            base=-(P - 1), pattern=[[-P, P]],
                            channel_multiplier=1)
    # qsel[s][p, m] = 1 iff (p==s, m==0) : qprev row s into row 0
    qsel = []
    for s in range(NSLOT):
        qs_t = const.tile([NSLOT, P], DT, tag=f"qsel{s}", name=f"qsel{s}")
        nc.gpsimd.memset(qs_t[:], 0.0)
        nc.gpsimd.affine_select(out=qs_t[:], in_=qs_t[:],
                                compare_op=Alu.not_equal, fill=1.0,
                                base=-s, pattern=[[-NSLOT, P]],
                                channel_multiplier=1)
        qsel.append(qs_t)
    ones1 = const.tile([P, 1], DT, tag="ones1")
    nc.gpsimd.memset(ones1[:], 1.0)

    wkup = const.tile([KV, QD], DT, tag="wkup")
    nc.gpsimd.dma_start(wkup[:], a["wkup"][:])
    # v_mid lives at base partition 32 inside kvmid; PE needs lhsT/rhs bases
    # to match, so W_vupT is loaded at partitions 32..63 as well.
    wvup64 = const.tile([64, C], DT, tag="wvup")
    nc.gpsimd.dma_start(wvup64[KV:64, :], a["wvup"][:])
    wvup = wvup64[KV:64, :]
    wqq = const.tile([P, 8, QD], DT, tag="wqq")
    nc.scalar.dma_start(wqq[:], wqqr[:])
    wkv = const.tile([P, 8, 64], DT, tag="wkv")
    nc.sync.dma_start(wkv[:], wkvr[:])
    xq_rep = const.tile([P, QD], DT, tag="xq_rep")
    nc.gpsimd.dma_start(xq_rep[:], a["xq_rep"][:])
    xk_rep = const.tile([P, QD], DT, tag="xk_rep")
    nc.gpsimd.dma_start(xk_rep[:], a["xk_rep"][:])
    maskall = const.tile([P, NMASK, CHUNK], DT, tag="maskall")
    nc.gpsimd.dma_start(maskall[:], maskr[:])
    gb = {}
    if apply_gb:
        for nm, d in [("gq", QD), ("bq", QD), ("gk", QD), ("bk", QD),
                      ("gv", C), ("bv", C)]:
            gb[nm] = const.tile([P, d], DT, tag=nm + "_rep", name=nm + "_rep")
            nc.gpsimd.dma_start(gb[nm][:], a[nm + "_rep"][:])

    loop = tc.For_i(0, nrep, 1) if nrep > 1 else None
    if loop is not None:
        loop.__enter__()

    # --- persistent strips ---
    kvmid = pers.tile([64, T], DT, tag="kvmid")       # [k_mid; v_mid]^T
    qraw = pers.tile([P, NQT, QD], DT, tag="qraw")    # canonical q tiles
    qprev = pers.tile([NSLOT, QD], DT, tag="qprev")   # chunk-boundary q rows
    kk = pers.tile([P, NT, QD], DT, tag="kk")         # k chain, [T, QD] tiles
    vv = pers.tile([P, NT, C], DT, tag="vv")          # v chain, [T, C] tiles
    kT = pers.tile([P, 2, T], DT, tag="kT")           # k^T for attention
    qT = pers.tile([P, 2, TQ], DT, tag="qT")          # q^T for attention

    gq, bq = (gb.get("gq"), gb.get("bq"))
    gk, bk = (gb.get("gk"), gb.get("bk"))
    gv, bv = (gb.get("gv"), gb.get("bv"))

    i32 = mybir.dt.int32

    def rsqrt_dve(x_ap, w, pool, nm, eng=None):
        # In-place x <- rsqrt(x + EPS) on DVE (or GpSimd): magic-constant
        # seed + 2 Newton iterations. Keeps Sqrt off ACT, whose function
        # table would need a ~1.3us reload to switch away from tanh/exp.
        eng = eng or nc.vector
        eng.tensor_scalar_add(out=x_ap, in0=x_ap, scalar1=EPS)
        yi = pool.tile([P, w], i32, tag=nm + "yi", name=nm + "yi")
        eng.tensor_scalar(out=yi[:], in0=x_ap.bitcast(i32),
                          scalar1=1, scalar2=None,
                          op0=Alu.arith_shift_right)
        eng.tensor_scalar(out=yi[:], in0=yi[:], scalar1=-1,
                          scalar2=0x5F3759DF, op0=Alu.mult,
                          op1=Alu.add)
        y = yi[:].bitcast(f32)
        t2 = pool.tile([P, w], f32, tag=nm + "t2", name=nm + "t2")
        for _ in range(1):
            eng.tensor_tensor(out=t2[:], in0=y, in1=y, op=Alu.mult)
            eng.tensor_tensor(out=t2[:], in0=t2[:], in1=x_ap,
                              op=Alu.mult)
            eng.tensor_scalar(out=t2[:], in0=t2[:], scalar1=-0.5,
                              scalar2=1.5, op0=Alu.mult, op1=Alu.add)
            eng.tensor_tensor(out=y, in0=y, in1=t2[:], op=Alu.mult)
        eng.tensor_copy(out=x_ap, in_=y)

    # ---------------- Phase A: kv_mid + q projections ----------------
    with (tc.tile_pool(name="xin", bufs=(4 if bf16 else 2)) as xin,
          tc.tile_pool(name="ps_a", bufs=2, space="PSUM") as ps_a):
        for tb in range(T // 512):
            xt = xin.tile([P, 8, 512], DT, tag="xt")
            if tb == 0:
                # split the first block across both HWDGE rings so the
                # leading kv matmuls start ~1.5us sooner
                nc.sync.dma_start(xt[:, 0:4, :],
                                  xTr[:, 0:4, 0:512])
                nc.scalar.dma_start(xt[:, 4:8, :],
                                    xTr[:, 4:8, 0:512])
            else:
                nc.sync.dma_start(xt[:],
                                  xTr[:, :, tb * 512:(tb + 1) * 512])
            kvps = ps_a.tile([64, 512], f32, tag="kvps")
            for cc in range(8):
                nc.tensor.matmul(kvps[:], wkv[:, cc, :], xt[:, cc, :],
                                 start=(cc == 0), stop=(cc == 7))
            nc.scalar.copy(kvmid[:, tb * 512:(tb + 1) * 512], kvps[:])

        for th in range(2):     # canonical q in two 512-token halves
            xqt = xin.tile([P, 8, 512], DT, tag="xt", name="xqt")
            # scalar ring: runs in parallel with the xt stream on sync
            nc.scalar.dma_start(xqt[:],
                                xqTr[:, :, th * 512:(th + 1) * 512])
            for j in range(4):
                tt = th * 4 + j
                qps = ps_a.tile([P, QD], f32, tag="qps")
                for cc in range(8):
                    nc.tensor.matmul(qps[:], xqt[:, cc, j * P:(j + 1) * P],
                                     wqq[:, cc, :],
                                     start=(cc == 0), stop=(cc == 7))
                nc.scalar.copy(qraw[:, tt, :], qps[:])

        xqp = xin.tile([P, 8, NSLOT], DT, tag="xqp")
        nc.sync.dma_start(xqp[:], xqpr[:])
        qpps = ps_a.tile([NSLOT, QD], f32, tag="qpps")
        for cc in range(8):
            nc.tensor.matmul(qpps[:], xqp[:, cc, :], wqq[:, cc, :],
                             start=(cc == 0), stop=(cc == 7))
        nc.scalar.copy(qprev[:], qpps[:])

    # ---------------- Phase B: k up-proj + embeddings ----------------
    if phases < 2:
        if loop is not None:
            loop.__exit__(None, None, None)
        ctx.close()
        return
    # v-emb pools/prefetch declared early: group 0's DMAs issue at the
    # start of phase B so the first v tiles never wait on the embedding
    # stream; each group start prefetches the next (pool is double-buffered)
    vembp = ctx.enter_context(tc.tile_pool(name="vemb", bufs=2))
    vemb_q = []

    def vemb_fetch(g):
        e1 = vembp.tile([P, 4, C], DT, tag="vemb1", name=f"vemb1_{g}")
        nc.sync.dma_start(e1[:], vembr1[:, g * 4:(g + 1) * 4, :])
        e2 = vembp.tile([P, 4, C], DT, tag="vemb2", name=f"vemb2_{g}")
        nc.sync.dma_start(e2[:], vembr2[:, g * 4:(g + 1) * 4, :])
        vemb_q.append((e1, e2))

    with (tc.tile_pool(name="emb", bufs=3) as embp,
          tc.tile_pool(name="ps_b", bufs=2, space="PSUM") as ps_b):
        kemb_q = []

        def kemb_fetch(g):
            t = embp.tile([P, 4, QD], DT, tag="kemb", name=f"kemb{g}")
            nc.scalar.dma_start(t[:], kembr[:, g * 4:(g + 1) * 4, :])
            kemb_q.append(t)

        kemb_fetch(0)
        vemb_fetch(0)
        for g in range(NT // 4):
            if g < 3:
                kemb_fetch(g + 1)   # prefetch next group
            kemb = kemb_q.pop(0)
            for j in range(4):
                tt = g * 4 + j
                kps = ps_b.tile([P, QD], f32, tag="kps")
                nc.tensor.matmul(kps[:], kvmid[0:KV, tt * P:(tt + 1) * P],
                                 wkup[:], start=True, stop=True)
                nc.vector.tensor_tensor(out=kk[:, tt, :], in0=kps[:],
                                        in1=kemb[:, j, :], op=Alu.mult)

    # ------- v chain (emitted per tile, interleaved with q/k chains) -------
    vwork = ctx.enter_context(tc.tile_pool(name="vwork", bufs=4))
    vstate = {"v2_prev": None, "vemb1": None, "vemb2": None}

    def emit_v(tt, ps_ch):
        g, j = tt // 4, tt % 4
        if j == 0:
            if g < 3:
                vemb_fetch(g + 1)   # prefetch next group
            vstate["vemb1"], vstate["vemb2"] = vemb_q.pop(0)
        vemb1, vemb2 = vstate["vemb1"], vstate["vemb2"]
        v2_prev = vstate["v2_prev"]
        vps = ps_ch.tile([P, C], f32, tag="vps", bufs=1, name=f"vps{tt}")
        for ch in range(2):
            nc.tensor.matmul(vps[:, ch * 512:(ch + 1) * 512],
                             kvmid[KV:64, tt * P:(tt + 1) * P],
                             wvup[:, ch * 512:(ch + 1) * 512],
                             start=True, stop=True)
        vt = vwork.tile([P, C], DT, tag="vt", name=f"vt{tt}")
        nc.scalar.activation(vt[:], vps[:], Act.Tanh)
        v2 = vwork.tile([P, C], DT, tag="v2", name=f"v2_{tt}")
        nc.vector.tensor_tensor(out=vv[:, tt, :], in0=vt[:],
                                in1=vemb1[:, j, :], op=Alu.mult)
        nc.vector.tensor_tensor(out=v2[:], in0=vt[:],
                                in1=vemb2[:, j, :], op=Alu.mult)
        shps = ps_ch.tile([P, C], f32, tag="vshps", bufs=1, name=f"vsh{tt}")
        for ch in range(0, C, NMAX):
            ce = ch + NMAX
            nc.tensor.matmul(shps[:, ch:ce], ssup[:], v2[:, ch:ce],
                             start=True, stop=v2_prev is None)
            if v2_prev is not None:
                nc.tensor.matmul(shps[:, ch:ce], bnd[:],
                                 v2_prev[:, ch:ce],
                                 start=False, stop=True)
        nc.vector.tensor_tensor(out=vv[:, tt, :], in0=shps[:],
                                in1=vv[:, tt, :], op=Alu.add)
        vstate["v2_prev"] = v2
        scr = vwork.tile([P, C], DT, tag="vscr", name=f"vscr{tt}")
        ssq = vwork.tile([P, 1], f32, tag="vssq", name=f"vssq{tt}")
        nc.scalar.activation(scr[:], vv[:, tt, :], Act.Square,
                             accum_out=ssq[:])
        ssm = vwork.tile([P, 1], f32, tag="vssm", name=f"vssm{tt}")
        nc.scalar.activation(scr[:], vv[:, tt, :], Act.Copy,
                             accum_out=ssm[:])
        mv = vwork.tile([P, 2], f32, tag="vmv", name=f"vmv{tt}")
        nc.vector.tensor_scalar_mul(out=mv[:, 0:1],
                                    in0=ssm[:], scalar1=1.0 / C)
        msq = vwork.tile([P, 1], f32, tag="vmsq", name=f"vmsq{tt}")
        nc.vector.tensor_tensor(out=msq[:], in0=mv[:, 0:1],
                                in1=mv[:, 0:1], op=Alu.mult)
        nc.vector.tensor_scalar_mul(out=mv[:, 1:2],
                                    in0=ssq[:], scalar1=1.0 / C)
        nc.vector.tensor_tensor(out=mv[:, 1:2], in0=mv[:, 1:2],
                                in1=msq[:], op=Alu.subtract)
        rsqrt_dve(mv[:, 1:2], 1, vwork, "vrs")
        nc.vector.tensor_scalar(out=vv[:, tt, :], in0=vv[:, tt, :],
                                scalar1=mv[:, 0:1],
                                scalar2=mv[:, 1:2],
                                op0=Alu.subtract, op1=Alu.mult)
        if gv is not None:
            nc.gpsimd.tensor_tensor(out=vv[:, tt, :], in0=vv[:, tt, :],
                                    in1=gv[:], op=Alu.mult)
            nc.gpsimd.tensor_tensor(out=vv[:, tt, :], in0=vv[:, tt, :],
                                    in1=bv[:], op=Alu.add)

    # ---------------- Phase C: shift + blend + layernorm ----------------
    if phases < 3:
        if loop is not None:
            loop.__exit__(None, None, None)
        ctx.close()
        return

    def blend_ln_t(src, dst, dstT, tt, coef, nm, prev_rhs, g, b,
                   ps_ch=None):
        shps = ps_ch.tile([P, QD], f32, tag="kqshps", bufs=2,
                          name=nm + f"ps{tt}")
        nc.tensor.matmul(shps[:], ssup[:], src[:, tt, :],
                         start=True, stop=prev_rhs is None)
        if prev_rhs is not None:
            nc.tensor.matmul(shps[:], prev_rhs[0], prev_rhs[1],
                             start=False, stop=True)
        tmp = shp.tile([P, QD], DT, tag=nm, name=nm + f"t{tt}")
        nc.vector.tensor_tensor(out=tmp[:], in0=shps[:], in1=src[:, tt, :],
                                op=Alu.subtract)
        nc.gpsimd.tensor_tensor(out=tmp[:], in0=tmp[:], in1=coef[:],
                                op=Alu.mult)
        nc.gpsimd.tensor_tensor(out=dst[:, tt, :], in0=src[:, tt, :],
                                in1=tmp[:], op=Alu.add)
        st = shp.tile([P, 6], f32, tag=nm + "st", name=nm + f"s{tt}")
        nc.vector.bn_stats(out=st[:], in_=dst[:, tt, :])
        mv = shp.tile([P, 2], f32, tag=nm + "mv", name=nm + f"m{tt}")
        nc.vector.bn_aggr(out=mv[:], in_=st[:])
        rsqrt_dve(mv[:, 1:2], 1, shp, nm + "rs")
        nc.vector.tensor_scalar(out=dst[:, tt, :], in0=dst[:, tt, :],
                                scalar1=mv[:, 0:1], scalar2=mv[:, 1:2],
                                op0=Alu.subtract, op1=Alu.mult)
        if g is not None:
            nc.gpsimd.tensor_tensor(out=dst[:, tt, :], in0=dst[:, tt, :],
                                    in1=g[:], op=Alu.mult)
            nc.gpsimd.tensor_tensor(out=dst[:, tt, :], in0=dst[:, tt, :],
                                    in1=b[:], op=Alu.add)
        for qc in range(2):
            tps = ps_ch.tile([P, P], DT, tag="tps", bufs=2)
            nc.tensor.transpose(tps[:], dst[:, tt, qc * P:(qc + 1) * P],
                                ident[:])
            nc.vector.tensor_copy(out=dstT[:, qc, tt * P:(tt + 1) * P],
                                  in_=tps[:])

    kf = pers.tile([P, NT, QD], DT, tag="kf")
    qf = pers.tile([P, NQT, QD], DT, tag="qf")
    # One shared PSUM scope for all three chains, emitted interleaved per
    # tile: sequential scopes would serialize the streams through PSUM
    # address reuse. Banks: vps 2 + vshps 2 + kqshps 2 + tps 2 = 8.
    with (tc.tile_pool(name="shp", bufs=4) as shp,
          tc.tile_pool(name="ps_ch", bufs=1, space="PSUM") as ps_ch):
        for tt in range(NT):
            emit_v(tt, ps_ch)
            if tt < NQT:
                prev = ((qsel[tt // 2], qprev[:]) if tt % 2 == 0
                        else (bnd, qraw[:, tt - 1, :]))
                blend_ln_t(qraw, qf, qT, tt, xq_rep, "qsh", prev, gq, bq,
                           ps_ch=ps_ch)
            prev = None if tt == 0 else (bnd, kk[:, tt - 1, :])
            blend_ln_t(kk, kf, kT, tt, xk_rep, "ksh", prev, gk, bk,
                       ps_ch=ps_ch)

    # ---------------- Phase D: attention ----------------
    if phases < 4:
        if loop is not None:
            loop.__exit__(None, None, None)
        ctx.close()
        return
    with (tc.tile_pool(name="att", bufs=6) as attp,
          tc.tile_pool(name="outs", bufs=2) as outsp,
          tc.tile_pool(name="ps_sc", bufs=2, space="PSUM") as ps_sc,
          tc.tile_pool(name="ps_out", bufs=1, space="PSUM") as ps_out,
          tc.tile_pool(name="ps_sum", bufs=1, space="PSUM") as ps_sum):
        for s in range(NSLOT):
            sums = [ps_sum.tile([P, 1], f32, tag=f"sums{i}",
                                name=f"sums_{s}_{i}") for i in range(2)]
            ops = [ps_out.tile([P, 512], f32, tag=f"o{i}{ch}",
                               name=f"ops_{s}_{i}{ch}")
                   for i in range(2) for ch in range(2)]
            # k-tiles processed in pairs: one [128, 2, 256] score block per
            # pair halves the tanh/exp/mask op count. Mask-needing k-tiles
            # are pair-aligned per slot by construction.
            for kp in range(R[s] // 2):
                sps = ps_sc.tile([P, 2, CHUNK], f32, tag="sps")
                for h in range(2):
                    kt = 2 * kp + h
                    for qc in range(2):
                        nc.tensor.matmul(
                            sps[:, h, :], kT[:, qc, kt * P:(kt + 1) * P],
                            qT[:, qc, s * CHUNK:(s + 1) * CHUNK],
                            start=(qc == 0), stop=(qc == 1))
                et = attp.tile([P, 2, CHUNK], DT, tag="et")
                nc.scalar.activation(et[:], sps[:], Act.Tanh,
                                     scale=1.0 / SCORE_SCALE)
                if (s, 2 * kp) in MASK_IDX:
                    mi = MASK_IDX[(s, 2 * kp)]
                    assert MASK_IDX[(s, 2 * kp + 1)] == mi + 1
                    nc.gpsimd.tensor_tensor(
                        out=et[:], in0=et[:],
                        in1=maskall[:, mi:mi + 2, :], op=Alu.add)
                ee = attp.tile([P, 2, CHUNK], DT, tag="ee")
                nc.scalar.activation(ee[:], et[:], Act.Exp, scale=CAP_SCALE)
                for h in range(2):
                    kt = 2 * kp + h
                    first, last = kt == 0, kt == R[s] - 1
                    for i in range(2):
                        nc.tensor.matmul(sums[i][:],
                                         ee[:, h, i * P:(i + 1) * P],
                                         ones1[:], start=first, stop=last)
                        for ch in range(2):
                            nc.tensor.matmul(
                                ops[2 * i + ch][:],
                                ee[:, h, i * P:(i + 1) * P],
                                vv[:, kt, ch * 512:(ch + 1) * 512],
                                start=first, stop=last)
            recip = attp.tile([P, 2], f32, tag="recip")
            for i in range(2):
                nc.vector.reciprocal(recip[:, i:i + 1], sums[i][:])
            for i in range(2):
                ot = outsp.tile([P, C], DT, tag="ot")
                for ch in range(2):
                    nc.vector.tensor_scalar_mul(
                        out=ot[:, ch * 512:(ch + 1) * 512],
                        in0=ops[2 * i + ch][:], scalar1=recip[:, i:i + 1])
                # scalar HWDGE ring is idle in phase D and has ~3x lower
                # fixed latency than SWDGE for these terminal stores
                nc.scalar.dma_start(
                    out_d[s * CHUNK + i * P:s * CHUNK + (i + 1) * P, :],
                    ot[:])

    if loop is not None:
        loop.__exit__(None, None, None)
    ctx.close()


_NC_CACHE = {}


def _input_specs(apply_gb, bf16):
    import concourse.mybir as mybir
    f32 = mybir.dt.float32
    DT = mybir.dt.bfloat16 if bf16 else f32
    specs = [
        ("xT", [C, T], DT), ("xqT", [C, TQ], DT),
        ("xqprevT", [C, NSLOT], DT),
        ("kemb", [T, QD], DT), ("vemb1", [T, C], DT),
        ("vemb2", [T, C], DT),
        ("wqq", [C, QD], DT), ("wkv", [C, 64], DT),
        ("wkup", [KV, QD], DT), ("wvup", [KV, C], DT),
        ("xq_rep", [P, QD], DT), ("xk_rep", [P, QD], DT),
        ("mask", [NMASK, P, CHUNK], DT),
    ]
    if apply_gb:
        specs += [("gq_rep", [P, QD], DT), ("bq_rep", [P, QD], DT),
                  ("gk_rep", [P, QD], DT), ("bk_rep", [P, QD], DT),
                  ("gv_rep", [P, C], DT), ("bv_rep", [P, C], DT)]
    return specs


def get_nc(apply_gb, bf16=True, nrep=1, phases=4):
    key = (bool(apply_gb), bool(bf16), int(nrep), int(phases))
    if key in _NC_CACHE:
        return _NC_CACHE[key]
    import concourse.mybir as mybir
    import concourse.tile as tile
    from concourse import bacc

    nc = bacc.Bacc("TRN2", target_bir_lowering=False, debug=False,
                   num_devices=N_CORES)
    a = {}
    for name, shape, dt in _input_specs(apply_gb, bf16):
        a[name] = nc.dram_tensor(name, shape, dt, kind="ExternalInput").ap()
    DTo = mybir.dt.bfloat16 if bf16 else mybir.dt.float32
    a["out"] = nc.dram_tensor("out", [TQ, C], DTo,
                              kind="ExternalOutput").ap()
    with tile.TileContext(nc) as tc:
        _build_program(nc, tc, a, apply_gb, bf16, nrep=nrep, phases=phases)
    nc.compile()
    _NC_CACHE[key] = nc
    return nc


def _parity_mask(parity):
    m = np.zeros((NMASK, P, CHUNK), np.float32)
    for (s, kt), mi in MASK_IDX.items():
        qs = CHUNKS[parity][s] * CHUNK
        kg = np.arange(P, dtype=np.int64)[:, None] + P * kt
        qg = np.arange(CHUNK, dtype=np.int64)[None, :] + qs
        m[mi] = np.where(qg >= kg, 0.0, NEG).astype(np.float32)
    return m


def make_in_maps(inputs, bf16=True):
    import ml_dtypes
    cdt = ml_dtypes.bfloat16 if bf16 else np.float32

    x = np.asarray(inputs["x"], np.float32)
    idx = np.asarray(inputs["idx"]).astype(np.int64)
    k_tab = np.asarray(inputs["k_emb_tab"], np.float32)
    v_tab = np.asarray(inputs["v_emb_tab"], np.float32)
    W_qq = np.asarray(inputs["W_qq"], np.float32)
    W_k = np.asarray(inputs["W_k"], np.float32)
    W_kup = np.asarray(inputs["W_kup"], np.float32)
    W_v = np.asarray(inputs["W_v"], np.float32)
    W_vup = np.asarray(inputs["W_vup"], np.float32)
    x_q = np.asarray(inputs["x_q"], np.float32).reshape(QD)
    x_k = np.asarray(inputs["x_k"], np.float32).reshape(QD)
    x_v = np.asarray(inputs["x_v"], np.float32).reshape(C)
    g_q = np.asarray(inputs["g_q"], np.float32).reshape(QD)
    b_q = np.asarray(inputs["b_q"], np.float32).reshape(QD)
    g_k = np.asarray(inputs["g_k"], np.float32).reshape(QD)
    b_k = np.asarray(inputs["b_k"], np.float32).reshape(QD)
    g_v = np.asarray(inputs["g_v"], np.float32).reshape(C)
    b_v = np.asarray(inputs["b_v"], np.float32).reshape(C)

    apply_gb = not (np.all(g_q == 1) and np.all(b_q == 0)
                    and np.all(g_k == 1) and np.all(b_k == 0)
                    and np.all(g_v == 1) and np.all(b_v == 0))

    k_emb = k_tab[idx]          # [B, T, QD]
    v_emb = v_tab[idx]          # [B, T, C]
    vemb1 = [np.ascontiguousarray(v_emb[b] * (1.0 - x_v)).astype(cdt)
             for b in range(B)]
    vemb2 = [np.ascontiguousarray(v_emb[b] * x_v).astype(cdt)
             for b in range(B)]

    def cvt(arr):
        return np.ascontiguousarray(arr).astype(cdt)

    shared = {
        "wqq": cvt(W_qq.T),
        "wkv": cvt(np.concatenate([W_k, W_v], 0).T),
        "wkup": cvt(W_kup.T),
        "wvup": cvt(W_vup.T),
        "xq_rep": cvt(np.broadcast_to(x_q, (P, QD))),
        "xk_rep": cvt(np.broadcast_to(x_k, (P, QD))),
    }
    if apply_gb:
        for nm, v in [("gq", g_q), ("bq", b_q), ("gk", g_k), ("bk", b_k)]:
            shared[nm + "_rep"] = cvt(np.broadcast_to(v, (P, QD)))
        for nm, v in [("gv", g_v), ("bv", b_v)]:
            shared[nm + "_rep"] = cvt(np.broadcast_to(v, (P, C)))

    pmask = [_parity_mask(0).astype(cdt), _parity_mask(1).astype(cdt)]
    in_maps = []
    for c in range(N_CORES):
        b, parity = c // 2, c % 2
        chunks = CHUNKS[parity]
        cols = np.concatenate([np.arange(ch * CHUNK, (ch + 1) * CHUNK)
                               for ch in chunks])
        xqprev = np.zeros((NSLOT, C), np.float32)
        for j, ch in enumerate(chunks):
            if ch > 0:
                xqprev[j] = x[b, ch * CHUNK - 1]
        m = dict(shared)
        m.update(
            xT=cvt(x[b].T), xqT=cvt(x[b][cols].T),
            xqprevT=cvt(xqprev.T),
            kemb=cvt(k_emb[b]),
            vemb1=vemb1[b], vemb2=vemb2[b],
            mask=pmask[parity],
        )
        in_maps.append(m)
    return in_maps, apply_gb


def assemble_output(results):
    out = np.empty((B, T, C), np.float32)
    for c in range(N_CORES):
        oc = np.asarray(results[c]["out"]).astype(np.float32)
        for j, ch in enumerate(CHUNKS[c % 2]):
            out[c // 2, ch * CHUNK:(ch + 1) * CHUNK] = \
                oc[j * CHUNK:(j + 1) * CHUNK]
    return out


BF16 = True


def kernel(**inputs):
    from concourse.bass_utils import run_bass_kernel_spmd
    in_maps, apply_gb = make_in_maps(inputs, bf16=BF16)
    nc = get_nc(apply_gb, bf16=BF16)
    res = run_bass_kernel_spmd(nc, in_maps, core_ids=list(range(N_CORES)))
    return assemble_output(res.results)

